# revision 14
# baseline (speedup 1.0000x reference)
"""Trainium2 Bass kernel for a message-aggregation (single-query attention) block.

Computation (per batch row b):
    Q = A @ Wq.T ; K = M @ Wk.T ; V = M @ Wv.T
    attn = softmax(Q . K / sqrt(D))
    out = sigmoid(A @ Wg.T + bg) * LN(attn-weighted V @ Wo.T + bo)

Host-side algebraic restructuring (exact up to fp reassociation):
    scores[b,n] = Qt[b] . M[b,n]          with Qt = A @ (Wq.T @ Wk) / sqrt(D)
    agg[b]      = (sum_n attn[b,n] M[b,n]) @ (Wo @ Wv).T + bo
    out         = gg * LN_nogamma(agg) + gb   with gg = gate*gamma, gb = gate*beta
Qt, gg, gb are cheap O(B*D) host precomputes; K and V are never materialized.
Messages are shipped to the device in fp16 (halves the HBM stream; score and
attention-weighted sums still accumulate in fp32).

Device dataflow, per 128-row batch tile, in 8-message groups:
  scores: groups 0-1 via one 2x-mode fp16 multiply + ScalarE
          Copy-with-accumulate reductions; groups 2-3 via fused DVE
          scalar_tensor_tensor (multiply + fp32 sum in one pass) — balances
          DVE vs ScalarE, and group 0's messages are the first DMA so ScalarE
          starts early.
  per group: exp (no max subtraction; scores ~ N(0,1)) + its sum in one
          ScalarE op, diag(exp) built by one GPSIMD affine_select (single
          read stream), 8 TensorE diag matmuls accumulate into PSUM.
  tail:   1/sumexp folded into the PSUM evacuation, transpose, (Wo Wv).T
          matmul + bias, LayerNorm, gate multiply (GPSIMD), fp16 store
          (upcast on host). Tails run at elevated scheduler priority one
          tile behind the score pass.

Sharding: pure data parallel over the batch dim across 8 cores; the small
512x512 weights are replicated. Messages stream on the sync/HWDGE ring; the
small inputs go through SWDGE so no engine queues behind the message stream.
"""

import math
from contextlib import ExitStack

import numpy as np

import concourse.bacc as bacc
import concourse.bass as bass
import concourse.mybir as mybir
import concourse.tile as tile
from concourse.bass_utils import run_bass_kernel_spmd
from concourse.masks import make_identity

B = 4096
N = 32
D = 512
NCORES = 8
BLOC = B // NCORES  # 512
P = 128
NT = BLOC // P  # 4 batch tiles per core
KT = D // P  # 4 contraction tiles
U = 16  # messages per DMA unit
NU = N // U  # 2 units per tile
G = 8  # messages per exp/diag/matmul group
NG = N // G  # 4 groups per tile
SCALE = math.sqrt(D)
LN_EPS = 1e-5

F32 = mybir.dt.float32
F16 = mybir.dt.float16
ALU = mybir.AluOpType
ACTF = mybir.ActivationFunctionType


def broadcast_mid(ap2d, count):
    """[P, X] AP -> [P, count, X] AP with a step-0 middle dim."""
    return bass.AP(
        tensor=ap2d.tensor,
        offset=ap2d.offset,
        ap=[ap2d.ap[0], [0, count], ap2d.ap[1]],
    )


def broadcast_last(ap2d, count):
    """[P, X] AP -> [P, X, count] AP with a step-0 last dim."""
    return bass.AP(
        tensor=ap2d.tensor,
        offset=ap2d.offset,
        ap=[ap2d.ap[0], ap2d.ap[1], [0, count]],
    )


def build_program():
    nc = bacc.Bacc(
        "TRN2",
        target_bir_lowering=False,
        debug=False,
        num_devices=NCORES,
    )

    m_d = nc.dram_tensor("m", [BLOC, N, D], F16, kind="ExternalInput")
    # aux rows: 0=qt, 1=gg, 2=gb
    aux_d = nc.dram_tensor("aux", [BLOC, 3, D], F16, kind="ExternalInput")
    wvo_d = nc.dram_tensor("wvo", [D, D], F16, kind="ExternalInput")  # (Wo @ Wv).T
    ones_d = nc.dram_tensor("ones", [1, D], F16, kind="ExternalInput")
    bo_d = nc.dram_tensor("bo", [1, D], F16, kind="ExternalInput")
    out_d = nc.dram_tensor("out", [BLOC, D], F16, kind="ExternalOutput")

    with tile.TileContext(nc) as tc, ExitStack() as ctx:
        consts = ctx.enter_context(tc.tile_pool(name="consts", bufs=1))
        mpool = ctx.enter_context(tc.tile_pool(name="mpool", bufs=6))
        auxp = ctx.enter_context(tc.tile_pool(name="auxp", bufs=NT))
        wts = ctx.enter_context(tc.tile_pool(name="wts", bufs=1))
        prodp = ctx.enter_context(tc.tile_pool(name="prodp", bufs=4))
        prod16p = ctx.enter_context(tc.tile_pool(name="prod16p", bufs=4))
        dumpp = ctx.enter_context(tc.tile_pool(name="dumpp", bufs=2))
        scp = ctx.enter_context(tc.tile_pool(name="scp", bufs=2))
        diagp = ctx.enter_context(tc.tile_pool(name="diagp", bufs=6))
        smalls = ctx.enter_context(tc.tile_pool(name="smalls", bufs=3))
        bigp = ctx.enter_context(tc.tile_pool(name="bigp", bufs=2))
        lhstp = ctx.enter_context(tc.tile_pool(name="lhstp", bufs=2))
        outp = ctx.enter_context(tc.tile_pool(name="outp", bufs=2))
        ps_m = ctx.enter_context(tc.tile_pool(name="ps_m", bufs=2, space="PSUM"))
        ps_t = ctx.enter_context(tc.tile_pool(name="ps_t", bufs=2, space="PSUM"))
        ps_a = ctx.enter_context(tc.tile_pool(name="ps_a", bufs=2, space="PSUM"))

        # ---- message stream: sync/HWDGE ring carries only messages + the
        # output stores; unit 0 of each tile feeds the ScalarE score path so
        # its reductions start as soon as the first unit lands.
        m_tiles = []  # [tile][unit]
        for i in range(NT):
            row = []
            for u in range(NU):
                t = mpool.tile([P, U, D], F16, tag="m")
                nc.sync.dma_start(
                    out=t[:],
                    in_=m_d[i * P : (i + 1) * P, u * U : (u + 1) * U, :],
                )
                row.append(t)
            m_tiles.append(row)

        # ---- small inputs via SWDGE (GPSIMD) so neither HWDGE ring stalls
        aux_t = []
        for i in range(NT):
            t = auxp.tile([P, 3, D], F16, tag="aux")
            nc.gpsimd.dma_start(out=t[:], in_=aux_d[i * P : (i + 1) * P, :, :])
            aux_t.append(t)

        wvo_t = wts.tile([P, KT, D], F16)
        nc.gpsimd.dma_start(
            out=wvo_t[:],
            in_=bass.AP(
                tensor=wvo_d[:, :].tensor,
                offset=0,
                ap=[[D, P], [P * D, KT], [1, D]],
            ),
        )
        ones_row = consts.tile([1, D], F16)
        nc.gpsimd.dma_start(out=ones_row[:], in_=ones_d[:, :])
        bo_row = consts.tile([1, D], F16)
        nc.gpsimd.dma_start(out=bo_row[:], in_=bo_d[:, :])

        # ---- constants ------------------------------------------------
        ident = consts.tile([P, P], F16)
        make_identity(nc, ident[:])
        eps_t = consts.tile([P, 1], F32)
        nc.vector.memset(eps_t[:], LN_EPS)

        st = [dict() for _ in range(NT)]  # per-tile state

        def qt_ap(i):
            return aux_t[i][:, 0, :]

        def emit_group(i, g):
            """Scores + exp + diag + weighted-sum matmuls for one 8-message
            group. Groups 0-1 (unit 0): ScalarE reduction path; groups 2-3
            (unit 1): fused DVE path."""
            sc = st[i]["sc"]
            u, base = (0, 0) if g < 2 else (1, U)
            mt = m_tiles[i][u]
            if g < 2:
                goff = (g % 2) * G
                p16 = prod16p.tile([P, G, D], F16, tag="prod16", name="p16")
                eng = nc.vector if g == 0 else nc.gpsimd
                eng.tensor_mul(
                    p16[:], mt[:, goff : goff + G, :], broadcast_mid(qt_ap(i), G)
                )
                for j in range(G):
                    dump = dumpp.tile([P, D], F16, tag="dump", name="dump")
                    nc.scalar.activation(
                        dump[:],
                        p16[:, j, :],
                        ACTF.Copy,
                        accum_out=sc[:, g * G + j : g * G + j + 1],
                    )
            else:
                goff = (g % 2) * G
                for j in range(G):
                    prod = prodp.tile([P, D], F16, tag="prod", name="prod")
                    nc.vector.scalar_tensor_tensor(
                        out=prod[:],
                        in0=mt[:, goff + j, :],
                        scalar=0.0,
                        in1=qt_ap(i),
                        op0=ALU.bypass,
                        op1=ALU.mult,
                        accum_out=sc[:, g * G + j : g * G + j + 1],
                    )
            if g % 2 == 1:
                # unit complete: exp + diag + weighted-sum matmuls for 16 msgs
                # (exp in two halves so the first half unblocks diag+PE early)
                lo = (g - 1) * G
                nc.scalar.activation(
                    st[i]["expd"][:, lo : lo + G],
                    sc[:, lo : lo + G],
                    ACTF.Exp,
                )
                nc.scalar.activation(
                    st[i]["expd"][:, lo + G : lo + U],
                    sc[:, lo + G : lo + U],
                    ACTF.Exp,
                )
                # dg[p, n, j] = (p == j) ? expd[p, n] : 0 — one GPSIMD pass
                dg = diagp.tile([P, U, P], F16, tag="diag", name="dg")
                nc.gpsimd.affine_select(
                    out=dg[:],
                    in_=broadcast_last(st[i]["expd"][:, lo : lo + U], P),
                    compare_op=ALU.is_equal,
                    fill=0.0,
                    base=0,
                    pattern=[[0, U], [-1, P]],
                    channel_multiplier=1,
                )
                if g == 1:
                    st[i]["pm"] = ps_m.tile([P, D], F32, tag="pm", name="pm")
                pm = st[i]["pm"]
                for j in range(U):
                    n = lo + j
                    nc.tensor.matmul(
                        pm[:],
                        lhsT=dg[:, j, :],
                        rhs=mt[:, j, :],
                        start=(n == 0),
                        stop=(n == N - 1),
                    )

        def emit_scores(i):
            st[i]["sc"] = scp.tile([P, N], F32, tag="sc", name="sc")
            st[i]["expd"] = scp.tile([P, N], F16, tag="expd", name="expd")
            for g in range(NG):
                emit_group(i, g)

        def emit_tail(i):
            sumexp = smalls.tile([P, 1], F32, tag="sumexp", name="sumexp")
            nc.vector.tensor_reduce(
                sumexp[:], st[i]["expd"][:], axis=mybir.AxisListType.X, op=ALU.add
            )
            rsum = smalls.tile([P, 1], F32, tag="rsum", name="rsum")
            nc.vector.reciprocal(rsum[:], sumexp[:])
            # fold 1/sum(exp) into the PSUM evacuation
            magg = bigp.tile([P, D], F16, tag="magg", name="magg")
            nc.scalar.mul(magg[:], st[i]["pm"][:], rsum[:, 0:1])

            # transpose m_agg so it can be the stationary operand
            pt = ps_t.tile([P, KT, P], F16, tag="pt", name="pt")
            for j in range(KT):
                nc.tensor.transpose(pt[:, j, :], magg[:, j * P : (j + 1) * P], ident[:])
            maggT = lhstp.tile([P, KT, P], F16, tag="lhst", name="maggT")
            for j in range(KT):
                nc.scalar.copy(maggT[:, j, :], pt[:, j, :])

            # agg = m_agg @ (Wo Wv).T + bo
            pa = ps_a.tile([P, D], F32, tag="pa", name="pa")
            for j in range(KT):
                nc.tensor.matmul(
                    pa[:],
                    lhsT=maggT[:, j, :],
                    rhs=wvo_t[:, j, :],
                    start=(j == 0),
                    stop=False,
                )
            nc.tensor.matmul(
                pa[:],
                lhsT=ones_row[:, 0:P],
                rhs=bo_row[:],
                start=False,
                stop=True,
            )

            # LayerNorm over d (gamma/beta folded into gg/gb on host)
            stats = smalls.tile([P, nc.vector.BN_STATS_DIM], F32, tag="stats", name="stats")
            nc.vector.bn_stats(stats[:], pa[:])
            mv = smalls.tile([P, nc.vector.BN_AGGR_DIM], F32, tag="mv", name="mv")
            nc.vector.bn_aggr(mv[:], stats[:])
            sq = smalls.tile([P, 1], F32, tag="sq", name="sq")
            nc.scalar.activation(sq[:], mv[:, 1:2], ACTF.Sqrt, bias=eps_t[:, 0:1])
            rstd = smalls.tile([P, 1], F32, tag="rstd", name="rstd")
            nc.vector.reciprocal(rstd[:], sq[:])
            negmr = smalls.tile([P, 1], F32, tag="negmr", name="negmr")
            nc.vector.tensor_scalar(
                negmr[:],
                mv[:, 0:1],
                scalar1=rstd[:, 0:1],
                scalar2=-1.0,
                op0=ALU.mult,
                op1=ALU.mult,
            )
            normed = outp.tile([P, D], F16, tag="normed", name="normed")
            nc.scalar.activation(
                normed[:], pa[:], ACTF.Identity, bias=negmr[:, 0:1], scale=rstd[:, 0:1]
            )

            # out = gg*normed + gb on GPSIMD (DVE/ACT are the hot engines)
            o = outp.tile([P, D], F16, tag="out", name="o")
            nc.gpsimd.tensor_mul(o[:], normed[:], aux_t[i][:, 1, :])
            nc.gpsimd.tensor_add(o[:], o[:], aux_t[i][:, 2, :])
            return o

        for s in range(NT + 1):
            if 0 <= s - 1 < NT:
                # the tail is a long cross-engine chain of small ops; elevated
                # priority lets the scheduler slot it between score ops
                with tc.high_priority():
                    o = emit_tail(s - 1)
                # store on the otherwise-idle scalar HWDGE ring at normal
                # priority (on the sync ring it would block later message
                # DMAs in the FIFO)
                nc.scalar.dma_start(
                    out=out_d[(s - 1) * P : s * P, :], in_=o[:]
                )
            if s < NT:
                emit_scores(s)

    nc.compile()
    return nc


_CACHED_NC = None


def _get_program():
    global _CACHED_NC
    if _CACHED_NC is None:
        _CACHED_NC = build_program()
    return _CACHED_NC


def make_in_maps(agent_hidden, messages, Wq, Wk, Wv, Wo, bo, gamma, beta, Wg, bg):
    A = np.asarray(agent_hidden, np.float32)
    M = np.asarray(messages, np.float32)
    wq = np.asarray(Wq, np.float64)
    wk = np.asarray(Wk, np.float64)
    wv = np.asarray(Wv, np.float64)
    wo = np.asarray(Wo, np.float64)

    wqk = ((wq.T @ wk) / SCALE).astype(np.float32)
    qt = (A @ wqk).astype(np.float16)
    gate = 1.0 / (
        1.0 + np.exp(-(A @ np.asarray(Wg, np.float32).T + np.asarray(bg, np.float32)))
    )
    gg = (gate * np.asarray(gamma, np.float32)).astype(np.float16)
    gb = (gate * np.asarray(beta, np.float32)).astype(np.float16)
    aux = np.ascontiguousarray(np.stack([qt, gg, gb], axis=1))  # [B, 3, D]
    wvo = np.ascontiguousarray((wo @ wv).T.astype(np.float16))
    m16 = M.astype(np.float16)

    bo_r = np.asarray(bo, np.float32).astype(np.float16).reshape(1, D)
    ones_r = np.ones((1, D), np.float16)

    in_maps = []
    for c in range(NCORES):
        sl = slice(c * BLOC, (c + 1) * BLOC)
        in_maps.append(
            {
                "m": np.ascontiguousarray(m16[sl]),
                "aux": aux[c * BLOC : (c + 1) * BLOC],
                "wvo": wvo,
                "ones": ones_r,
                "bo": bo_r,
            }
        )
    return in_maps


def kernel(**inputs) -> np.ndarray:
    nc = _get_program()
    in_maps = make_in_maps(**inputs)
    res = run_bass_kernel_spmd(nc, in_maps, core_ids=list(range(NCORES)))
    return np.concatenate([r["out"] for r in res.results], axis=0).astype(np.float32)
